# revision 1
# baseline (speedup 1.0000x reference)
"""Trainium2 Bass kernel for DSS-GIN conv (gnn_message_passing).

Strategy (8 NeuronCores, B=128 subgraphs sharded 16/core):
  - h = MLP_t(X) computed per-core in transposed space via PE matmuls (fp32r).
  - Pooled node branch: per-core partial max over local subgraphs, AllReduce(max)
    across cores (split into 4 column groups, pipelined behind stage A), then
    nodex = MLP_n(xmax) replicated on every core.
  - Message passing ret1 + broadcast nodex2 folded into ONE dense matmul:
      out[b] = S^T @ (h[b] + nodex)   where S[j,k] = #edges j->k  (built on host
    from edge_index, streamed from DRAM as 128x128 tiles).
  - All matmuls in float32r (full PE rate at free>=256, ~12-bit mantissa).
"""
import sys
sys.path.insert(0, '/opt/trn_rl_repo')

import numpy as np


def _ensure_ntff_hook_module():
    """Provide antenv.axon_hooks if the image lacks it (needed only when
    BASS_TRACE=1 requests NTFF profiling through run_bass_kernel_spmd)."""
    try:
        import antenv.axon_hooks  # noqa: F401
        return
    except Exception:
        pass
    import contextlib, ctypes, os, types

    mod = types.ModuleType("antenv.axon_hooks")
    state = {"hook": None, "tried": False}
    so_path = "/opt/axon/libaxon_pjrt.so"

    def _make_hook(path):
        lib = ctypes.CDLL(path)
        if not hasattr(lib, "axon_start_nrt_profile"):
            return None
        lib.axon_start_nrt_profile.argtypes = [
            ctypes.POINTER(ctypes.c_int64), ctypes.c_size_t]
        lib.axon_start_nrt_profile.restype = ctypes.c_int64
        lib.axon_stop_nrt_profile.argtypes = [ctypes.c_char_p]
        lib.axon_stop_nrt_profile.restype = ctypes.c_int64

        @contextlib.contextmanager
        def _hook(output_dir, device_ids):
            import jax
            jax.devices()
            if device_ids:
                ids = (ctypes.c_int64 * len(device_ids))(*device_ids)
                rc = lib.axon_start_nrt_profile(ids, len(device_ids))
            else:
                rc = lib.axon_start_nrt_profile(None, 0)
            if rc != 0:
                raise RuntimeError(f"axon_start_nrt_profile rc={rc}")
            try:
                yield
            finally:
                n = lib.axon_stop_nrt_profile(str(output_dir).encode())
                if n < 0:
                    raise RuntimeError(f"axon_stop_nrt_profile rc={n}")
                print(f"profile: {n} file(s) written to {output_dir}")

        return _hook

    def get_axon_ntff_profile_hook():
        if state["hook"] is None and not state["tried"]:
            state["tried"] = True
            if os.path.exists(so_path):
                try:
                    state["hook"] = _make_hook(so_path)
                except Exception:
                    state["hook"] = None
        return state["hook"]

    def set_axon_ntff_profile_hook(hook):
        state["hook"] = hook
        state["tried"] = True

    mod.get_axon_ntff_profile_hook = get_axon_ntff_profile_hook
    mod.set_axon_ntff_profile_hook = set_axon_ntff_profile_hook
    sys.modules["antenv.axon_hooks"] = mod


_ensure_ntff_hook_module()

NCORES = 8
B, N, D, E = 128, 2048, 64, 32768
BL = B // NCORES          # 16 subgraphs per core
NT = N // 128             # 16 node tiles
NCHUNK = 512              # bn-chunk: 4 node tiles for one subgraph
NG = N // NCHUNK          # 4 chunks per subgraph

_BUILD_CACHE = {}
LAST_RESULTS = None


def _build(zero_bias=False):
    key = ("nc", zero_bias)
    if key in _BUILD_CACHE:
        return _BUILD_CACHE[key]
    import concourse.bacc as bacc
    import concourse.tile as tile
    from concourse import mybir
    dt = mybir.dt
    f32, f32r = dt.float32, dt.float32r
    Relu = mybir.ActivationFunctionType.Relu
    Alu = mybir.AluOpType

    nc = bacc.Bacc("TRN2", target_bir_lowering=False, debug=False)

    Xc = nc.dram_tensor("Xc", [BL, N, D], f32, kind="ExternalInput").ap()
    St = nc.dram_tensor("St", [NT, 128, NT, 128], dt.uint8, kind="ExternalInput").ap()
    W1n = nc.dram_tensor("W1n", [D, D], f32, kind="ExternalInput").ap()
    B1n = nc.dram_tensor("B1n", [D, 1], f32, kind="ExternalInput").ap()
    W2n = nc.dram_tensor("W2n", [D, D], f32, kind="ExternalInput").ap()
    B2n = nc.dram_tensor("B2n", [D, 1], f32, kind="ExternalInput").ap()
    W1DD = nc.dram_tensor("W1DD", [128, 128], f32, kind="ExternalInput").ap()
    W2DD = nc.dram_tensor("W2DD", [128, 128], f32, kind="ExternalInput").ap()
    B1DD = nc.dram_tensor("B1DD", [128, 1], f32, kind="ExternalInput").ap()
    B2DD = nc.dram_tensor("B2DD", [128, 1], f32, kind="ExternalInput").ap()
    Ident = nc.dram_tensor("Ident", [128, 128], f32, kind="ExternalInput").ap()
    Out = nc.dram_tensor("Out", [BL, N, D], f32, kind="ExternalOutput").ap()

    with tile.TileContext(nc) as tc:
        with tc.tile_pool(name="const", bufs=1) as constp, \
             tc.tile_pool(name="resident", bufs=1) as resp, \
             tc.tile_pool(name="osb", bufs=3) as osbp, \
             tc.tile_pool(name="sslf", bufs=2) as sslfp, \
             tc.tile_pool(name="dram", bufs=1, space="DRAM") as dram:

            # ---- constants ----
            ident = constp.tile([128, 128], f32)
            nc.sync.dma_start(ident[:], Ident[:])
            ident_r = constp.tile([64, 64], f32r)
            nc.vector.tensor_copy(ident_r[:], ident[:64, :64])
            ident_r128 = constp.tile([128, 128], f32r)
            nc.vector.tensor_copy(ident_r128[:], ident[:])

            wdd_f32 = constp.tile([128, 2 * 128], f32)
            nc.sync.dma_start(wdd_f32[:, 0:128], W1DD[:])
            nc.sync.dma_start(wdd_f32[:, 128:256], W2DD[:])
            wdd_r = constp.tile([128, 2 * 128], f32r)
            nc.vector.tensor_copy(wdd_r[:], wdd_f32[:])
            w1dd, w2dd = wdd_r[:, 0:128], wdd_r[:, 128:256]
            bdd = constp.tile([128, 2], f32)
            nc.sync.dma_start(bdd[:, 0:1], B1DD[:])
            nc.sync.dma_start(bdd[:, 1:2], B2DD[:])
            b1dd, b2dd = bdd[:, 0:1], bdd[:, 1:2]

            w_f32 = constp.tile([D, 2 * D], f32)
            nc.sync.dma_start(w_f32[:, 0 * D:1 * D], W1n[:])
            nc.sync.dma_start(w_f32[:, 1 * D:2 * D], W2n[:])
            w_r = constp.tile([D, 2 * D], f32r)
            nc.vector.tensor_copy(w_r[:], w_f32[:])
            w1n, w2n = w_r[:, 0 * D:1 * D], w_r[:, 1 * D:2 * D]
            biases = constp.tile([D, 2], f32)
            nc.sync.dma_start(biases[:, 0:1], B1n[:])
            nc.sync.dma_start(biases[:, 1:2], B2n[:])
            b1n, b2n = biases[:, 0:1], biases[:, 1:2]

            # ---- resident tensors ----
            # h split by b-half for clean pass-1/pass-2 dependencies
            h_half = [
                resp.tile([128, NT, BL // 2, D], f32r, name=f"hh{i}")
                for i in range(2)
            ]

            xpn = resp.tile([128, NT, D], f32)     # partial max, natural layout
            xmn = resp.tile([128, NT, D], f32)     # global max, natural layout
            xmaxT = resp.tile([64, N], f32r)       # global max, transposed
            nodex = resp.tile([128, NT, D], f32)

            cin = dram.tile([128, NT * D], f32)
            crs = dram.tile([16, NT * D], f32)
            cout = dram.tile([128, NT * D], f32)

            def pass_c(psC, kts):
                """stage C: both b-halves for each kt"""
                for kt in kts:
                    sslu = sslfp.tile([128, NT, 128], dt.uint8, tag="sslu")
                    nc.sync.dma_start(sslu[:], St[kt])
                    ssl = sslfp.tile([128, NT, 128], f32r, tag="ssl")
                    nc.vector.tensor_copy(ssl[:], sslu[:])
                    for H in range(2):
                        pc = psC.tile([128, 512], f32, tag="pc")
                        for jt in range(NT):
                            nc.tensor.matmul(
                                pc[:],
                                ssl[:, jt, :],
                                h_half[H][:, jt, :, :].rearrange("p b d -> p (b d)"),
                                start=(jt == 0), stop=(jt == NT - 1))
                        osb = osbp.tile([128, 512], f32, tag="osb")
                        nc.scalar.activation(osb[:], pc[:], Relu)
                        nc.sync.dma_start(
                            Out[H * 8:(H + 1) * 8,
                                kt * 128:(kt + 1) * 128, :].rearrange(
                                "b p d -> p b d"),
                            osb[:].rearrange("p (b d) -> p b d", d=D))

            # ============ stage A: MLP_t in pair-chunks, by b-half ============
            with tc.tile_pool(name="xn", bufs=28) as xnp, \
                 tc.tile_pool(name="xt", bufs=4) as xtp, \
                 tc.tile_pool(name="mid", bufs=4) as midp, \
                 tc.tile_pool(name="htt", bufs=4) as http, \
                 tc.tile_pool(name="nbr", bufs=2) as nbrp, \
                 tc.tile_pool(name="psA", bufs=2, space="PSUM") as psA:

                def chunk(p, g):
                    b0 = 2 * p
                    H = p // 4
                    xn = xnp.tile([128, 4, 2, D], f32r, tag="xn")
                    for j in range(2):
                        nc.sync.dma_start(
                            xn[:, :, j, :],
                            Xc[b0 + j, g * NCHUNK:(g + 1) * NCHUNK, :].rearrange(
                                "(t p) d -> p t d", p=128).bitcast(f32r),
                        )
                    gsl = slice(4 * g, 4 * g + 4)
                    # partial max in natural layout (runs right behind the DMA)
                    if p == 0:
                        nc.vector.tensor_tensor(
                            xpn[:, gsl, :], xn[:, :, 0, :].bitcast(f32), xn[:, :, 1, :].bitcast(f32), Alu.max)
                    else:
                        nc.vector.tensor_tensor(
                            xpn[:, gsl, :], xpn[:, gsl, :], xn[:, :, 0, :].bitcast(f32), Alu.max)
                        nc.vector.tensor_tensor(
                            xpn[:, gsl, :], xpn[:, gsl, :], xn[:, :, 1, :].bitcast(f32), Alu.max)
                    # transpose -> [128 (b,d), 512 n]
                    tp = psA.tile([128, NCHUNK], f32r, tag="tp")
                    for t in range(4):
                        nc.tensor.transpose(
                            tp[:, t * 128:(t + 1) * 128],
                            xn[:, t, :, :].rearrange("p b d -> p (b d)"),
                            ident_r128[:])
                    xt = xtp.tile([128, NCHUNK], f32r, tag="xt")
                    nc.vector.tensor_copy(xt[:], tp[:])
                    # L1/L2 with block-diagonal weights (2 subgraphs at once)
                    l1p = psA.tile([128, NCHUNK], f32, tag="l1")
                    nc.tensor.matmul(l1p[:], w1dd, xt[:], start=True, stop=True)
                    mid = midp.tile([128, NCHUNK], f32r, tag="mid")
                    if zero_bias:
                        nc.vector.tensor_scalar_max(mid[:], l1p[:], 0.0)
                    else:
                        nc.scalar.activation(mid[:], l1p[:], Relu, bias=b1dd)
                    l2p = psA.tile([128, NCHUNK], f32, tag="l2")
                    nc.tensor.matmul(l2p[:], w2dd, mid[:], start=True, stop=True)
                    htt = http.tile([128, NCHUNK], f32r, tag="htt")
                    nc.scalar.activation(htt[:], l2p[:], Relu, bias=b2dd)
                    # transpose back: [128 n, (2b x 64d)] slabs into h_half
                    htp = psA.tile([128, 4, 2, D], f32r, tag="htp")
                    for t in range(4):
                        nc.tensor.transpose(
                            htp[:, t, :, :].rearrange("p b d -> p (b d)"),
                            htt[:, t * 128:(t + 1) * 128], ident_r128[:])
                    nc.scalar.activation(
                        h_half[H][:, gsl, (b0 % 8):(b0 % 8) + 2, :], htp[:], Relu)

                # ---- b-half 0 then b-half 1 chunk pipelines ----
                for g in range(NG):
                    for p in range(4):
                        chunk(p, g)
                for g in range(NG):
                    for p in range(4, 8):
                        chunk(p, g)

                # ---- pooled branch (high prio: jumps engine queues) ----
                nc.sync.dma_start(cin[:], xpn[:].rearrange("p a b -> p (a b)"))
                nc.gpsimd.collective_compute(
                    "ReduceScatter",
                    Alu.max,
                    replica_groups=[list(range(NCORES))],
                    ins=[cin[:].opt()],
                    outs=[crs[:].opt()],
                )
                nc.gpsimd.collective_compute(
                    "AllGather",
                    Alu.bypass,
                    replica_groups=[list(range(NCORES))],
                    ins=[crs[:].opt()],
                    outs=[cout[:].opt()],
                )
                nc.sync.dma_start(xmn[:].rearrange("p a b -> p (a b)"), cout[:])
                # transpose xmn -> xmaxT
                for q in range(NG):
                    tpn = psA.tile([64, NCHUNK], f32, tag="l1")
                    for t in range(4):
                        nc.tensor.transpose(
                            tpn[:, t * 128:(t + 1) * 128],
                            xmn[:, 4 * q + t, :], ident[:])
                    nc.vector.tensor_copy(
                        xmaxT[:, q * NCHUNK:(q + 1) * NCHUNK], tpn[:])
                # MLP_n
                for q in range(NG):
                    qs = slice(q * NCHUNK, (q + 1) * NCHUNK)
                    l1pn = psA.tile([64, NCHUNK], f32, tag="l1")
                    nc.tensor.matmul(l1pn[:], w1n, xmaxT[:, qs], start=True, stop=True)
                    midn = nbrp.tile([64, NCHUNK], f32r, tag="midn")
                    nc.scalar.activation(midn[:], l1pn[:], Relu, bias=b1n)
                    l2pn = psA.tile([64, NCHUNK], f32, tag="l2")
                    nc.tensor.matmul(l2pn[:], w2n, midn[:], start=True, stop=True)
                    httn = nbrp.tile([64, NCHUNK], f32r, tag="httn")
                    nc.scalar.activation(httn[:], l2pn[:], Relu, bias=b2n)
                    htpn = psA.tile([128, 4, D], f32r, tag="htp")
                    for t in range(4):
                        nc.tensor.transpose(
                            htpn[:, t, :],
                            httn[:, t * 128:(t + 1) * 128], ident_r[:])
                    nc.vector.tensor_copy(nodex[:, 4 * q:4 * q + 4, :], htpn[:])

                # h' = h + nodex for b-half 0
                for jt in range(NT):
                    nc.vector.tensor_tensor(
                        h_half[0][:, jt],
                        h_half[0][:, jt],
                        nodex[:, jt, None, :].broadcast_to((128, BL // 2, D)),
                        Alu.add)

                for jt in range(NT):
                    nc.vector.tensor_tensor(
                        h_half[1][:, jt],
                        h_half[1][:, jt],
                        nodex[:, jt, None, :].broadcast_to((128, BL // 2, D)),
                        Alu.add)

            # =================== stage C ===================
            with tc.tile_pool(name="psC", bufs=4, space="PSUM") as psC:
                pass_c(psC, range(NT))

    nc.compile()
    _BUILD_CACHE[key] = nc
    return nc


def kernel(X, edge_index, W1t, b1t, W2t, b2t, W1n, b1n, W2n, b2n):
    global LAST_RESULTS
    from concourse.bass_utils import run_bass_kernel_spmd

    zb = all(
        float(np.abs(np.asarray(v)).max()) == 0.0
        for v in (b1t, b2t, b1n, b2n))
    nc = _build(zero_bias=zb)

    X = np.ascontiguousarray(X, dtype=np.float32)
    # dense adjacency S[src, dst] = edge count, tiled [kt, jt, 128, 128]
    S = np.zeros((N, N), dtype=np.int32)
    np.add.at(S, (edge_index[0].astype(np.int64), edge_index[1].astype(np.int64)), 1)
    assert S.max() < 256
    St = np.ascontiguousarray(
        S.reshape(NT, 128, NT, 128).transpose(2, 1, 0, 3).astype(np.uint8))

    common = {
        "St": St,
        "W1n": np.ascontiguousarray(W1n, np.float32),
        "B1n": np.ascontiguousarray(b1n, np.float32).reshape(D, 1),
        "W2n": np.ascontiguousarray(W2n, np.float32),
        "B2n": np.ascontiguousarray(b2n, np.float32).reshape(D, 1),
        "W1DD": np.block([
            [np.asarray(W1t, np.float32), np.zeros((D, D), np.float32)],
            [np.zeros((D, D), np.float32), np.asarray(W1t, np.float32)]]),
        "W2DD": np.block([
            [np.asarray(W2t, np.float32), np.zeros((D, D), np.float32)],
            [np.zeros((D, D), np.float32), np.asarray(W2t, np.float32)]]),
        "B1DD": np.concatenate([np.asarray(b1t, np.float32).ravel()] * 2).reshape(128, 1),
        "B2DD": np.concatenate([np.asarray(b2t, np.float32).ravel()] * 2).reshape(128, 1),
        "Ident": np.eye(128, dtype=np.float32),
    }
    in_maps = [
        {"Xc": np.ascontiguousarray(X[c * BL:(c + 1) * BL]), **common}
        for c in range(NCORES)
    ]
    import os as _os
    _tc = list(range(NCORES)) if _os.environ.get("BASS_TRACE_ALL") else None
    res = run_bass_kernel_spmd(nc, in_maps, list(range(NCORES)), trace_cores=_tc)
    LAST_RESULTS = res
    out = np.empty((B, N, D), dtype=np.float32)
    for c in range(NCORES):
        out[c * BL:(c + 1) * BL] = res.results[c]["Out"]
    return out



# revision 12
# speedup vs baseline: 1.5478x; 1.5478x over previous
"""Trainium2 Bass kernel for DSS-GIN conv (gnn_message_passing).

Strategy (8 NeuronCores, B=128 subgraphs sharded 16/core):
  - X is pre-transposed + bf16-cast on host: XT[pair, (b2,d), n]. Stage A
    (tuplewise MLP) runs in bf16 with block-diagonal weights (2 subgraphs
    per 128-partition tile); h is stored transposed-back in fp8 e4m3.
  - Dense adjacency S (built on host from edge_index) is encoded e4m3 on
    host (counts are small ints, exact) and kept resident in SBUF.
  - Stage C (message passing) is ONE fp8 DoubleRow matmul chain per dst
    tile: pc[dst,(b,d)] = sum_j S[j,dst] * h[j,(b,d)] at 2x PE rate,
    contraction 256/step.
  - Pooled branch: partial max over local subgraphs on DVE (bf16),
    ONE AllReduce(max) overlapped with stage A tail / stage C head, then
    nodex = MLP_n(xmax), oN = S^T nodex (fp8 DoubleRow), folded into the
    output during PSUM eviction (DVE add) so stage C never waits on the
    collective.
"""
import sys
sys.path.insert(0, '/opt/trn_rl_repo')

import numpy as np
import ml_dtypes


def _ensure_ntff_hook_module():
    """Provide antenv.axon_hooks if the image lacks it (needed only when
    BASS_TRACE=1 requests NTFF profiling through run_bass_kernel_spmd)."""
    try:
        import antenv.axon_hooks  # noqa: F401
        return
    except Exception:
        pass
    import contextlib, ctypes, os, types

    mod = types.ModuleType("antenv.axon_hooks")
    state = {"hook": None, "tried": False}
    so_path = "/opt/axon/libaxon_pjrt.so"

    def _make_hook(path):
        lib = ctypes.CDLL(path)
        if not hasattr(lib, "axon_start_nrt_profile"):
            return None
        lib.axon_start_nrt_profile.argtypes = [
            ctypes.POINTER(ctypes.c_int64), ctypes.c_size_t]
        lib.axon_start_nrt_profile.restype = ctypes.c_int64
        lib.axon_stop_nrt_profile.argtypes = [ctypes.c_char_p]
        lib.axon_stop_nrt_profile.restype = ctypes.c_int64

        @contextlib.contextmanager
        def _hook(output_dir, device_ids):
            import jax
            jax.devices()
            if device_ids:
                ids = (ctypes.c_int64 * len(device_ids))(*device_ids)
                rc = lib.axon_start_nrt_profile(ids, len(device_ids))
            else:
                rc = lib.axon_start_nrt_profile(None, 0)
            if rc != 0:
                raise RuntimeError(f"axon_start_nrt_profile rc={rc}")
            try:
                yield
            finally:
                n = lib.axon_stop_nrt_profile(str(output_dir).encode())
                if n < 0:
                    raise RuntimeError(f"axon_stop_nrt_profile rc={n}")
                print(f"profile: {n} file(s) written to {output_dir}")

        return _hook

    def get_axon_ntff_profile_hook():
        if state["hook"] is None and not state["tried"]:
            state["tried"] = True
            if os.path.exists(so_path):
                try:
                    state["hook"] = _make_hook(so_path)
                except Exception:
                    state["hook"] = None
        return state["hook"]

    def set_axon_ntff_profile_hook(hook):
        state["hook"] = hook
        state["tried"] = True

    mod.get_axon_ntff_profile_hook = get_axon_ntff_profile_hook
    mod.set_axon_ntff_profile_hook = set_axon_ntff_profile_hook
    sys.modules["antenv.axon_hooks"] = mod


_ensure_ntff_hook_module()

NCORES = 8
B, N, D, E = 128, 2048, 64, 32768
BL = B // NCORES          # 16 subgraphs per core
NPAIR = BL // 2           # 8 subgraph pairs per core
NT = N // 128             # 16 node tiles
NCHUNK = 512              # node chunk for stage A
NG = N // NCHUNK          # 4 chunk groups

_BUILD_CACHE = {}
LAST_RESULTS = None


def _build():
    if "nc" in _BUILD_CACHE:
        return _BUILD_CACHE["nc"]
    import concourse.bacc as bacc
    import concourse.tile as tile
    from concourse import mybir
    dt = mybir.dt
    f32, bf16, f8 = dt.float32, dt.bfloat16, dt.float8e4
    Relu = mybir.ActivationFunctionType.Relu
    Alu = mybir.AluOpType
    DR = mybir.MatmulPerfMode.DoubleRow

    nc = bacc.Bacc("TRN2", target_bir_lowering=False, debug=False)

    # X transposed per pair: [pair, (b2 x 64d), n]
    XT = nc.dram_tensor("XT", [NPAIR, 128, N], bf16, kind="ExternalInput").ap()
    # S counts as e4m3: [kt, p, tp, i, dst]  (src j = 256*tp + 128*i + p)
    St = nc.dram_tensor("St", [NT, 128, NPAIR, 2, 128], f8, kind="ExternalInput").ap()
    W1DD = nc.dram_tensor("W1DD", [128, 128], bf16, kind="ExternalInput").ap()
    W2DD = nc.dram_tensor("W2DD", [128, 128], bf16, kind="ExternalInput").ap()
    B1DD = nc.dram_tensor("B1DD", [128, 1], f32, kind="ExternalInput").ap()
    B2DD = nc.dram_tensor("B2DD", [128, 1], f32, kind="ExternalInput").ap()
    W1N = nc.dram_tensor("W1N", [D, D], bf16, kind="ExternalInput").ap()
    W2N = nc.dram_tensor("W2N", [D, D], bf16, kind="ExternalInput").ap()
    B1N = nc.dram_tensor("B1N", [D, 1], f32, kind="ExternalInput").ap()
    B2N = nc.dram_tensor("B2N", [D, 1], f32, kind="ExternalInput").ap()
    IdentBF = nc.dram_tensor("IdentBF", [128, 128], bf16, kind="ExternalInput").ap()
    Out = nc.dram_tensor("Out", [BL, N, D], f32, kind="ExternalOutput").ap()

    with tile.TileContext(nc) as tc:
        with tc.tile_pool(name="const", bufs=1) as constp, \
             tc.tile_pool(name="resident", bufs=1) as resp, \
             tc.tile_pool(name="osb", bufs=4) as osbp, \
             tc.tile_pool(name="dram", bufs=1, space="DRAM") as dram:

            # ---- constants ----
            ident = constp.tile([128, 128], bf16)
            nc.sync.dma_start(ident[:], IdentBF[:])
            wdd = constp.tile([128, 2 * 128], bf16)
            nc.sync.dma_start(wdd[:, 0:128], W1DD[:])
            nc.sync.dma_start(wdd[:, 128:256], W2DD[:])
            w1dd, w2dd = wdd[:, 0:128], wdd[:, 128:256]
            bdd = constp.tile([128, 2], f32)
            nc.sync.dma_start(bdd[:, 0:1], B1DD[:])
            nc.sync.dma_start(bdd[:, 1:2], B2DD[:])
            b1dd, b2dd = bdd[:, 0:1], bdd[:, 1:2]
            wn = constp.tile([D, 2 * D], bf16)
            nc.sync.dma_start(wn[:, 0:D], W1N[:])
            nc.sync.dma_start(wn[:, D:2 * D], W2N[:])
            w1n, w2n = wn[:, 0:D], wn[:, D:2 * D]
            bn = constp.tile([D, 2], f32)
            nc.sync.dma_start(bn[:, 0:1], B1N[:])
            nc.sync.dma_start(bn[:, 1:2], B2N[:])
            b1n, b2n = bn[:, 0:1], bn[:, 1:2]

            # ---- resident tensors ----
            Sfull = resp.tile([128, NT, NPAIR, 2, 128], f8)    # 32KB/part
            h_full = resp.tile([128, NT, BL, D], f8)           # 16KB/part
            xpn2 = resp.tile([128, N], bf16)   # per-pair-lane local max, transposed
            xm2 = resp.tile([D, 2, N], bf16)   # global max halves, both at base 0
            xmaxT = resp.tile([D, N], bf16)    # global max, transposed
            nodex8 = resp.tile([128, NT, D], f8)               # MLP_n out, natural

            cin = dram.tile([128, N], bf16)
            cout = dram.tile([128, N], bf16)

            # ============ stage A: MLP_t in bf16, h -> fp8 ============
            with tc.tile_pool(name="xt", bufs=8) as xtp, \
                 tc.tile_pool(name="mid", bufs=3) as midp, \
                 tc.tile_pool(name="htt", bufs=3) as http, \
                 tc.tile_pool(name="psA", bufs=2, space="PSUM") as psA:

                def chunk(p, g):
                    gsl = slice(g * NCHUNK, (g + 1) * NCHUNK)
                    xt = xtp.tile([128, NCHUNK], bf16, tag="xt")
                    nc.sync.dma_start(xt[:], XT[p, :, gsl])
                    # partial max over local subgraph pairs (transposed layout;
                    # lanes 0:64 accumulate even subgraphs, 64:128 odd)
                    if p == 0:
                        nc.vector.tensor_copy(xpn2[:, gsl], xt[:])
                    else:
                        nc.vector.tensor_tensor(
                            xpn2[:, gsl], xpn2[:, gsl], xt[:], Alu.max)
                    l1p = psA.tile([128, NCHUNK], f32, tag="l1")
                    nc.tensor.matmul(l1p[:], w1dd, xt[:], start=True, stop=True)
                    mid = midp.tile([128, NCHUNK], bf16, tag="mid")
                    nc.scalar.activation(mid[:], l1p[:], Relu, bias=b1dd)
                    l2p = psA.tile([128, NCHUNK], f32, tag="l2")
                    nc.tensor.matmul(l2p[:], w2dd, mid[:], start=True, stop=True)
                    htt = http.tile([128, NCHUNK], bf16, tag="htt")
                    nc.scalar.activation(htt[:], l2p[:], Relu, bias=b2dd)
                    # transpose back to natural layout, evict to fp8 h
                    htp = psA.tile([128, 4, 2, D], bf16, tag="htp")
                    for t in range(4):
                        nc.tensor.transpose(
                            htp[:, t, :, :].rearrange("p b d -> p (b d)"),
                            htt[:, t * 128:(t + 1) * 128], ident[:])
                    nc.vector.tensor_copy(
                        h_full[:, 4 * g:4 * g + 4, 2 * p:2 * p + 2, :], htp[:])

                for g in range(NG):
                    for p in range(NPAIR):
                        chunk(p, g)
                    # stream in 4 S tiles per group (behind the X loads)
                    for kk in range(4):
                        kt = 4 * g + kk
                        nc.sync.dma_start(Sfull[:, kt], St[kt])

                # ---- pooled-branch collective (overlaps stage C head) ----
                nc.sync.dma_start(cin[:], xpn2[:])
                nc.gpsimd.collective_compute(
                    "AllReduce",
                    Alu.max,
                    replica_groups=[list(range(NCORES))],
                    ins=[cin[:].opt()],
                    outs=[cout[:].opt()],
                )
                # land both partition halves at base partition 0, then max
                nc.sync.dma_start(xm2[:, 0, :], cout[0:D, :])
                nc.sync.dma_start(xm2[:, 1, :], cout[D:128, :])
                nc.vector.tensor_tensor(
                    xmaxT[:], xm2[:, 0, :], xm2[:, 1, :], Alu.max)

            # =================== stage C + nodex branch ===================
            KT_PRE = 3  # dst tiles computed before nodex is ready
            with tc.tile_pool(name="psC", bufs=3, space="PSUM") as psC, \
                 tc.tile_pool(name="psN", bufs=1, space="PSUM") as psN, \
                 tc.tile_pool(name="nbr", bufs=2) as nbrp:

                oNsb = nbrp.tile([128, KT_PRE, D], f32, tag="oN")

                def mm_c(kt):
                    pc = psC.tile([128, BL, D], f32, tag="pc")
                    for tp in range(NPAIR):
                        for H in range(2):
                            nc.tensor.matmul(
                                pc[:, 8 * H:8 * H + 8, :].rearrange(
                                    "p b d -> p (b d)"),
                                Sfull[:, kt, tp],
                                h_full[:, 2 * tp:2 * tp + 2, 8 * H:8 * H + 8, :]
                                .rearrange("p i b d -> p i (b d)"),
                                start=(tp == 0), stop=(tp == NPAIR - 1),
                                perf_mode=DR)
                    return pc

                def evict_c(kt, pc, fold):
                    osb = osbp.tile([128, BL, D], f32, tag="osb")
                    if fold:
                        # fold S^T nodex in during eviction
                        nc.vector.tensor_tensor(
                            osb[:], pc[:],
                            oNsb[:, kt, None, :].broadcast_to((128, BL, D)),
                            Alu.add)
                    else:
                        nc.scalar.activation(osb[:], pc[:], Relu)
                    nc.sync.dma_start(
                        Out[:, kt * 128:(kt + 1) * 128, :].rearrange(
                            "b p d -> p b d"),
                        osb[:])

                # stage C matmuls for the first tiles run while the collective
                # completes; their evictions are emitted after oN is computed
                pcs = [mm_c(kt) for kt in range(KT_PRE)]

                # ---- nodex branch: MLP_n(xmax) -> nodex8 (natural fp8) ----
                for q in range(4):
                    qs = slice(q * 512, (q + 1) * 512)
                    l1n = psN.tile([D, 512], f32, tag="n")
                    nc.tensor.matmul(l1n[:], w1n, xmaxT[:, qs], start=True, stop=True)
                    midn = nbrp.tile([D, 512], bf16, tag="midn")
                    nc.scalar.activation(midn[:], l1n[:], Relu, bias=b1n)
                    l2n = psN.tile([D, 512], f32, tag="n")
                    nc.tensor.matmul(l2n[:], w2n, midn[:], start=True, stop=True)
                    httn = nbrp.tile([D, 512], bf16, tag="httn")
                    nc.scalar.activation(httn[:], l2n[:], Relu, bias=b2n)
                    # transpose to natural fp8 nodex
                    ntp = psN.tile([128, 4, D], bf16, tag="n")
                    for t in range(4):
                        nc.tensor.transpose(
                            ntp[:, t, :],
                            httn[:, t * 128:(t + 1) * 128],
                            ident[0:64, 0:64])
                    nc.scalar.activation(
                        nodex8[:, 4 * q:4 * q + 4, :], ntp[:], Relu)

                # ---- oN = S^T nodex for the first KT_PRE dst tiles ----
                # (one accumulation group at a time: PSUM start zeroes the
                # whole bank, so groups sharing a bank must not interleave)
                oNp = psN.tile([D, KT_PRE, 128], f32, tag="n")
                for kk in range(KT_PRE):
                    for tp in range(NPAIR):
                        nc.tensor.matmul(
                            oNp[:, kk, :],
                            nodex8[:, 2 * tp:2 * tp + 2, :],
                            Sfull[:, kk, tp],
                            start=(tp == 0), stop=(tp == NPAIR - 1),
                            perf_mode=DR)
                oNT_sb = nbrp.tile([D, KT_PRE, 128], bf16, tag="oNT")
                nc.scalar.activation(oNT_sb[:], oNp[:], Relu)
                ntp3 = psN.tile([128, KT_PRE, D], bf16, tag="n")
                for kk in range(KT_PRE):
                    nc.tensor.transpose(
                        ntp3[:, kk, :], oNT_sb[:, kk, :], ident[0:64, 0:64])
                nc.scalar.activation(oNsb[:], ntp3[:], Relu)

                # evict the deferred tiles now that oN exists
                for kt in range(KT_PRE):
                    evict_c(kt, pcs[kt], True)

                # ---- fold nodex into h for the remaining dst tiles ----
                for jt in range(NT):
                    nc.vector.tensor_tensor(
                        h_full[:, jt],
                        h_full[:, jt],
                        nodex8[:, jt, None, :].broadcast_to((128, BL, D)),
                        Alu.add)

                for kt in range(KT_PRE, NT):
                    evict_c(kt, mm_c(kt), False)

    nc.compile()
    _BUILD_CACHE["nc"] = nc
    return nc


def kernel(X, edge_index, W1t, b1t, W2t, b2t, W1n, b1n, W2n, b2n):
    global LAST_RESULTS
    from concourse.bass_utils import run_bass_kernel_spmd

    nc = _build()

    bf = ml_dtypes.bfloat16
    f8 = ml_dtypes.float8_e4m3

    X = np.asarray(X, dtype=np.float32)
    # XT[b, d, n] in bf16
    XT_all = np.ascontiguousarray(X.transpose(0, 2, 1)).astype(bf)

    # dense adjacency counts S[src, dst]
    S = np.zeros((N, N), dtype=np.int32)
    np.add.at(S, (edge_index[0].astype(np.int64), edge_index[1].astype(np.int64)), 1)
    # [kt, p, tp, i, dst] with src j = 256*tp + 128*i + p
    St8 = np.ascontiguousarray(
        S.reshape(NPAIR, 2, 128, NT, 128).transpose(3, 2, 0, 1, 4)
    ).astype(np.float32).astype(f8)

    W1t = np.asarray(W1t, np.float32)
    W2t = np.asarray(W2t, np.float32)
    zDD = np.zeros((D, D), np.float32)
    common = {
        "St": St8,
        "W1DD": np.block([[W1t, zDD], [zDD, W1t]]).astype(bf),
        "W2DD": np.block([[W2t, zDD], [zDD, W2t]]).astype(bf),
        "B1DD": np.concatenate([np.asarray(b1t, np.float32).ravel()] * 2).reshape(128, 1),
        "B2DD": np.concatenate([np.asarray(b2t, np.float32).ravel()] * 2).reshape(128, 1),
        "W1N": np.asarray(W1n, np.float32).astype(bf),
        "W2N": np.asarray(W2n, np.float32).astype(bf),
        "B1N": np.asarray(b1n, np.float32).reshape(D, 1),
        "B2N": np.asarray(b2n, np.float32).reshape(D, 1),
        "IdentBF": np.eye(128, dtype=np.float32).astype(bf),
    }
    in_maps = [
        {
            "XT": np.ascontiguousarray(
                XT_all[c * BL:(c + 1) * BL].reshape(NPAIR, 128, N)),
            **common,
        }
        for c in range(NCORES)
    ]
    import os as _os
    _tc = list(range(NCORES)) if _os.environ.get("BASS_TRACE_ALL") else None
    res = run_bass_kernel_spmd(nc, in_maps, list(range(NCORES)), trace_cores=_tc)
    LAST_RESULTS = res
    out = np.empty((B, N, D), dtype=np.float32)
    for c in range(NCORES):
        out[c * BL:(c + 1) * BL] = res.results[c]["Out"]
    return out
